# revision 30
# baseline (speedup 1.0000x reference)
"""ArcMargin head (ArcFace) distributed over 8 TRN2 NeuronCores.

Strategy (classification / tensor parallel), v11:
  - weight [C, D] sharded along C (12500 classes/core, padded to 12544);
    embeddings + labels replicated.  Weight is uploaded bf16 twice
    (transposed [D, CSP] for the matmul lhsT, natural [CSP, D] for the class
    norms); embeddings only once, transposed [D, B] (raw matmul rhs).
  - TRANSPOSED logits out[c, b] = 64 * (w_c . e_hat_b): classes sit on PSUM
    partitions, so both norms fold into PSUM evacuation: 1/||w_c|| is a
    per-partition scalar, 64/||e_b|| is the per-column tensor renb [128, B],
    built on device as ones[128,128] @ embt^2 (a K=128 matmul computes all
    column sums-of-squares broadcast to every partition) + sqrt + recip.
  - The first JD=8 chunks evacuate into SBUF staging with only the rn scale
    (renb is not ready yet) and are finalized a few chunks later - the
    TensorEngine never waits on the norm chain.  Scheduling rules learned
    from traces: cross-queue DMA dependencies act as per-queue completion
    barriers (DMA issue order = need order), a PE instruction must never
    wait on a SWDGE (gpsimd-queue) DMA semaphore, an indirect (SWDGE)
    gather must never be followed by writes to the tensor it reads (the
    WAR dependency bubbles the whole write queue behind it), and the
    finalize window j=9..23 runs the DMA queue at ~350 GB/s - adding any
    extra traffic there backlogs the weight prefetches and stalls the PE.
  - Output is bf16 (halves the dominant HBM write traffic; rel-err budget
    2e-2 >> bf16 noise).
  - ArcFace margin (v11): the host PERMUTES each core's class shard so that
    every target class (the ones needing the margin) sits in the first
    NG*128 positions = the first staged chunks.  Those chunks' rn-scaled
    staging values are copied to a dedicated DRAM buffer gsrc at j=1..4
    (a window where the store queue is otherwise idle), and renb row 0 is
    dumped to DRAM (4 KB) right after it's built.  At j=6/7 two tiny SWDGE
    gathers fetch the staged value and its renb factor for each target row;
    at j=10 the DVE reconstructs 64*cos = stg*renb, applies the phi
    formula, and ships 64*phi in a tiny tensor.  The whole margin chain
    lives in j<=11, touches nothing the main stream depends on, and
    NOTHING runs after the last chunk's store.  The host un-permutes
    during unshard (indexing only, ~500 rows).
  - Prologue (v11): embeddings stream in 4x512-column pieces interleaved
    with the first weight blocks so matmuls can start on a 0.75 MB
    prefix; ~36 dummy 128-column matmuls on the ones tile warm the PE HAM
    clock-gate (4/8 -> 8/8 after ~3.4us of activity) before the stream,
    and more dummies bridge chunk 0's DMA-paced gaps so the HAM MID
    window never re-throttles.
"""

import math
import sys

import numpy as np
import ml_dtypes

for _p in ("/opt/trn_rl_repo",):
    if _p not in sys.path:
        sys.path.append(_p)

import concourse.bass as bass
import concourse.tile as tile
from concourse import bacc
from concourse import mybir
from concourse.bass_utils import run_bass_kernel_spmd

SCALE = 64.0
MARGIN = 0.5
COS_M = math.cos(MARGIN)
SIN_M = math.sin(MARGIN)
TH = math.cos(math.pi - MARGIN)
MM = math.sin(math.pi - MARGIN) * MARGIN

B, D, C = 2048, 512, 100000
N_CORES = 8
CS = C // N_CORES          # 12500 real classes per core
CSP = 12544                # padded classes per core (98 * 128)
NJ = CSP // 128            # 98 class chunks
CB = 1792                  # weight-block width (7 blocks x 14 chunks)
NBLK = CSP // CB           # 7
JPB = CB // 128            # 14 chunks per block
OOB = 1 << 30              # gather offset sentinel for "not my row"
JD = 8                     # chunks evacuated to SBUF staging (pre-renb)
NG = 4                     # phi gather columns (target classes live in
                           # chunks 0..NG-1 after the host permutation)
NWARM = 36                 # HAM warm-up dummy matmuls (N=128 each)

NPBF = ml_dtypes.bfloat16

F32 = mybir.dt.float32
BF16 = mybir.dt.bfloat16
I32 = mybir.dt.int32
AF = mybir.ActivationFunctionType
ALU = mybir.AluOpType


def build_program(b=B, d=D, csp=CSP):
    """Build the (SPMD-uniform) single-core Bass program."""
    kc = d // 128          # 4 contraction chunks
    nc = bacc.Bacc()

    embt_d = nc.declare_dram_parameter("embt", [d, b], BF16, isOutput=False)
    wt_d = nc.declare_dram_parameter("wt", [d, csp], BF16, isOutput=False)
    wn_d = nc.declare_dram_parameter("wn", [csp, d], BF16, isOutput=False)
    soff_d = nc.declare_dram_parameter(
        "soff", [128, 2 * NG], I32, isOutput=False
    )
    # flat transposed output [c * B + b]
    out_d = nc.declare_dram_parameter("out", [csp * b, 1], BF16, isOutput=True)
    tv_d = nc.declare_dram_parameter("tv", [128, NG], F32, isOutput=True)
    # phi gather sources: staged (rn-scaled, pre-renb) copies of chunks
    # 0..NG-1, and one row of renb.  Both written in the idle early window
    # and only ever READ afterwards, so the gathers carry no WAR hazard
    # against anything.
    gsrc_d = nc.declare_dram_parameter(
        "gsrc", [NG * 128 * b, 1], BF16, isOutput=True
    )
    renb_d = nc.declare_dram_parameter("renbd", [b, 1], BF16, isOutput=True)

    with tile.TileContext(nc) as tc:
        with (
            tc.tile_pool(name="const", bufs=1) as constp,
            tc.tile_pool(name="persist", bufs=1) as persist,
            tc.tile_pool(name="wtp", bufs=3) as wtp,
            tc.tile_pool(name="wnp", bufs=3) as wnp,
            tc.tile_pool(name="scr", bufs=2) as scrp,
            tc.tile_pool(name="smp", bufs=4) as smp,
            tc.tile_pool(name="outp", bufs=4) as outp,
            tc.tile_pool(name="stg", bufs=1) as stgp,
            tc.tile_pool(name="cpsum", bufs=4, space="PSUM") as cpsum,
        ):
            zb = constp.tile([128, 1], F32, tag="zb")
            nc.vector.memset(zb[:], 0.0)
            epsb = constp.tile([128, 1], F32, tag="epsb")
            nc.vector.memset(epsb[:], 1e-24)
            s2b = constp.tile([128, 1], F32, tag="s2b")
            nc.vector.memset(s2b[:], SCALE * SCALE)
            onesb = constp.tile([128, 128], BF16, tag="onesb")
            nc.vector.memset(onesb[:], 1.0)

            embt = persist.tile([128, kc, b], BF16)     # e^T raw (matmul rhs)
            sqt = persist.tile([128, kc, b], BF16)      # embt^2
            renb = persist.tile([128, b], BF16)         # 64/||e_b|| bcast
            rsf = persist.tile([128, b], F32)           # 1/sum(e^2) scratch
            nsq = persist.tile([128, NJ], F32)          # per-class sum(w^2)
            nrm = persist.tile([128, NJ], F32)
            rn = persist.tile([128, NJ], F32)           # 1/||w_c||
            sq01 = persist.tile([128, b], BF16)         # embt^2 partial sum
            svec = persist.tile([128, NG], BF16)        # stg value of targets
            renbg = persist.tile([128, NG], BF16)       # renb value of targets
            tval = persist.tile([128, NG], F32)         # 64*phi, sorted
            sofft = persist.tile([128, 2 * NG], I32)
            stg = stgp.tile([128, JD, b], BF16)         # staged rn-scaled out

            outv = out_d[:].rearrange("(c b) o -> c (b o)", b=b)  # [csp, b]
            gsv = gsrc_d[:].rearrange("(c b) o -> c (b o)", b=b)  # [NG*128, b]

            # ---------------- DMA helpers ----------------
            wt_tiles = {}

            def wt_blk(blk):
                t = wtp.tile([128, kc, CB], BF16, tag="wt", name=f"wt_{blk}")
                nc.sync.dma_start(
                    out=t[:],
                    in_=wt_d[:, blk * CB:(blk + 1) * CB].rearrange(
                        "(k p) c -> p k c", p=128
                    ),
                )
                wt_tiles[blk] = t

            wn_tiles = {}

            def wn_g(g):
                r0 = g * 512
                ng = min(4, NJ - g * 4)
                t = wnp.tile([128, 4, d], BF16, tag="wn", name=f"wn_{g}")
                nc.sync.dma_start(
                    out=t[:, :ng, :],
                    in_=wn_d[r0:r0 + ng * 128, :].rearrange(
                        "(g2 p) dd -> p g2 dd", p=128
                    ),
                )
                wn_tiles[g] = t

            # ---------------- compute helpers ----------------
            def wnorm_chunk(c):
                sq = scrp.tile([128, d], BF16, tag="sqw")
                nc.scalar.activation(
                    out=sq[:], in_=wn_tiles[c // 4][:, c % 4, :], func=AF.Square,
                    bias=zb[:], accum_out=nsq[:, c:c + 1],
                )

            def rn_fin(g):
                s0 = g * 4
                s1 = min(s0 + 4, NJ)
                nc.scalar.activation(
                    out=nrm[:, s0:s1], in_=nsq[:, s0:s1], func=AF.Sqrt, bias=epsb[:]
                )
                nc.vector.reciprocal(out=rn[:, s0:s1], in_=nrm[:, s0:s1])

            def phi_gather():
                # all of gsrc is written by j=5; nothing writes it again, so
                # these gathers block nothing.  Per-column [128,1] offsets
                # (multi-column offset APs scramble the columns).
                for q in range(NG):
                    nc.gpsimd.indirect_dma_start(
                        out=svec[:, q:q + 1],
                        out_offset=None,
                        in_=gsrc_d[:],
                        in_offset=bass.IndirectOffsetOnAxis(
                            ap=sofft[:, q:q + 1], axis=0
                        ),
                        bounds_check=NG * 128 * b - 1,
                        oob_is_err=False,
                    )

            def renb_gather():
                for q in range(NG):
                    nc.gpsimd.indirect_dma_start(
                        out=renbg[:, q:q + 1],
                        out_offset=None,
                        in_=renb_d[:],
                        in_offset=bass.IndirectOffsetOnAxis(
                            ap=sofft[:, NG + q:NG + q + 1], axis=0
                        ),
                        bounds_check=b - 1,
                        oob_is_err=False,
                    )

            def phi_block():
                # 64*cos = staged value * renb factor (both gathered)
                sb = smp.tile([128, NG], F32, tag="sb")
                nc.vector.tensor_tensor(
                    out=sb[:], in0=svec[:, :], in1=renbg[:, :], op=ALU.mult
                )
                s2 = smp.tile([128, NG], F32, tag="s2")
                nc.vector.tensor_tensor(out=s2[:], in0=sb[:], in1=sb[:],
                                        op=ALU.mult)
                sn = smp.tile([128, NG], F32, tag="sn")
                # sin = sqrt(4096 - s^2); s^2 <= 4096 exactly (|cos| <= 1)
                nc.scalar.activation(
                    out=sn[:], in_=s2[:], func=AF.Sqrt, bias=s2b[:], scale=-1.0
                )
                pc = smp.tile([128, NG], F32, tag="pc")
                nc.vector.tensor_scalar_mul(out=pc[:], in0=sb[:], scalar1=COS_M)
                smt = smp.tile([128, NG], F32, tag="smt")
                nc.vector.tensor_scalar_mul(out=smt[:], in0=sn[:], scalar1=SIN_M)
                ph = smp.tile([128, NG], F32, tag="ph")
                nc.vector.tensor_tensor(
                    out=ph[:], in0=pc[:], in1=smt[:], op=ALU.subtract
                )
                eb = smp.tile([128, NG], F32, tag="eb")
                nc.vector.tensor_scalar_add(
                    out=eb[:], in0=sb[:], scalar1=-SCALE * MM
                )
                mk = smp.tile([128, NG], mybir.dt.uint8, tag="mk")
                nc.vector.tensor_scalar(
                    out=mk[:], in0=sb[:], scalar1=SCALE * TH, scalar2=None,
                    op0=ALU.is_gt,
                )
                nc.vector.select(
                    out=tval[:, :], mask=mk[:], on_true=ph[:], on_false=eb[:]
                )
                nc.sync.dma_start(out=tv_d[:, :], in_=tval[:, :])

            # ---------------- prologue (DMA order = ring order; order sets
            # the completion barrier each consumer waits on) ----------------
            def embt_q(qq):
                nc.sync.dma_start(
                    out=embt[:, :, qq * 512:(qq + 1) * 512],
                    in_=embt_d[:, qq * 512:(qq + 1) * 512].rearrange(
                        "(k p) c -> p k c", p=128
                    ),
                )

            embt_q(0)
            wt0a = wtp.tile([128, kc, 256], BF16, tag="wt0a")
            nc.sync.dma_start(
                out=wt0a[:],
                in_=wt_d[:, 0:256].rearrange("(k p) c -> p k c", p=128),
            )
            embt_q(1)
            wn_g(0)
            embt_q(2)
            embt_q(3)
            wt0b = wtp.tile([128, kc, 768], BF16, tag="wt0b")
            nc.sync.dma_start(
                out=wt0b[:],
                in_=wt_d[:, 256:1024].rearrange("(k p) c -> p k c", p=128),
            )
            wt0c = wtp.tile([128, kc, CB - 1024], BF16, tag="wt0c")
            nc.sync.dma_start(
                out=wt0c[:],
                in_=wt_d[:, 1024:CB].rearrange("(k p) c -> p k c", p=128),
            )
            wn_g(1)
            wn_g(2)
            wt_blk(1)
            wn_g(3)
            nc.sync.dma_start(out=sofft[:], in_=soff_d[:])

            # HAM warm-up: N=128 dummy matmuls on the ones tile keep the
            # PE busy from ~6.3us (constants ready) while the first real
            # operands stream in, flipping the clock gate to 8/8 before the
            # real matmul stream begins.  More dummies are interleaved into
            # chunk 0 (below) to bridge the DMA-paced gaps so the HAM MID
            # window never sees enough idle to re-throttle.
            wmps = cpsum.tile([128, 1024], F32, tag="mmps", name="warm")

            def dummy_mms(n):
                for _ in range(n):
                    nc.tensor.matmul(
                        out=wmps[:, 0:128], lhsT=onesb[:], rhs=onesb[:],
                        start=True, stop=True,
                    )

            dummy_mms(NWARM)

            wdone = 0
            while wdone < 8:
                wnorm_chunk(wdone)
                wdone += 1
                if wdone % 4 == 0:
                    rn_fin(wdone // 4 - 1)

            # ---------------- main loop over class chunks ----------------
            dsent = 0
            for j in range(NJ):
                blk, jj = divmod(j, JPB)
                # wn before the (big) wt block so a 1.75MB wt transfer never
                # delays the wnorm chain feeding the scalar-side evacuation
                if j % 4 == 0:
                    g = j // 4 + 4
                    if g * 4 < NJ:
                        wn_g(g)
                # two-block weight prefetch (bufs=3: cur, +1, +2 in flight)
                if j == 0:
                    wt_blk(2)
                elif jj == 0 and 1 <= blk <= NBLK - 3:
                    wt_blk(blk + 2)

                # staged chunks 0..NG-1 -> gsrc (idle store-queue window)
                if 1 <= j <= NG:
                    nc.sync.dma_start(
                        out=gsv[(j - 1) * 128:j * 128, :], in_=stg[:, j - 1, :]
                    )

                # embt^2 and its kc-axis tree reduction, split across DVE and
                # scalar so the ones-matmul only needs ONE contraction chunk
                # (4 matmuls of 512 cols instead of 16)
                if j == 2:
                    nc.vector.tensor_tensor(
                        out=sqt[:, 0, :], in0=embt[:, 0, :],
                        in1=embt[:, 0, :], op=ALU.mult,
                    )
                    nc.scalar.activation(
                        out=sqt[:, 1, :], in_=embt[:, 1, :], func=AF.Square,
                        bias=zb[:],
                    )
                if j == 3:
                    nc.vector.tensor_tensor(
                        out=sqt[:, 2, :], in0=embt[:, 2, :],
                        in1=embt[:, 2, :], op=ALU.mult,
                    )
                    nc.scalar.activation(
                        out=sqt[:, 3, :], in_=embt[:, 3, :], func=AF.Square,
                        bias=zb[:],
                    )
                    nc.vector.tensor_tensor(
                        out=sq01[:], in0=sqt[:, 0, :], in1=sqt[:, 1, :],
                        op=ALU.add,
                    )
                if j == 4:
                    # sqt slices 0/1 are dead once sq01 exists - reuse them
                    # as scratch for the rest of the tree (SBUF is tight)
                    nc.vector.tensor_tensor(
                        out=sqt[:, 0, :], in0=sqt[:, 2, :], in1=sqt[:, 3, :],
                        op=ALU.add,
                    )
                    nc.vector.tensor_tensor(
                        out=sqt[:, 1, :], in0=sq01[:], in1=sqt[:, 0, :],
                        op=ALU.add,
                    )
                if j == 5:
                    # renb = 64/||e_b|| on every partition: ones-matmul
                    # computes column sums of sq4 broadcast over partitions
                    for hh in range(2):
                        psr = cpsum.tile(
                            [128, 1024], F32, tag="mmps", name=f"ps_re{hh}"
                        )
                        for t2 in range(2):
                            nc.tensor.matmul(
                                out=psr[:, t2 * 512:(t2 + 1) * 512],
                                lhsT=onesb[:],
                                rhs=sqt[:, 1, (2 * hh + t2) * 512:
                                        (2 * hh + t2 + 1) * 512],
                                start=True,
                                stop=True,
                            )
                        # renb = 64/sqrt(ssq) = sqrt(4096 * (1/ssq)):
                        # fast approx reciprocal (18 bits >> bf16), then a
                        # scalar-engine sqrt with the scale folded in
                        nc.vector.reciprocal_approx_fast(
                            out=rsf[:, hh * 1024:(hh + 1) * 1024], in_=psr[:]
                        )
                        nc.scalar.activation(
                            out=renb[:, hh * 1024:(hh + 1) * 1024],
                            in_=rsf[:, hh * 1024:(hh + 1) * 1024],
                            func=AF.Sqrt, bias=epsb[:], scale=SCALE * SCALE,
                        )
                if j == 6:
                    # one row of renb to DRAM (4 KB) for the target gather
                    nc.sync.dma_start(
                        out=renb_d[:].rearrange("b o -> o b"),
                        in_=renb[0:1, :],
                    )
                if j == 7:
                    phi_gather()
                if j == 8:
                    renb_gather()
                if j == 30:
                    # late enough that the (slow, serialized) SWDGE gathers
                    # are long finished - the DVE queue must never block on
                    # them (head-of-line: every evac behind it would stall
                    # and backpressure PSUM into the PE)
                    phi_block()

                if blk == 0:
                    def lhs(k, jj=jj):
                        if jj < 2:
                            return wt0a[:, k, jj * 128:(jj + 1) * 128]
                        if jj < 8:
                            return wt0b[:, k, (jj - 2) * 128:(jj - 1) * 128]
                        return wt0c[:, k, (jj - 8) * 128:(jj - 7) * 128]
                else:
                    def lhs(k, jj=jj, cw=wt_tiles[blk]):
                        return cw[:, k, jj * 128:(jj + 1) * 128]

                staged = j < JD
                last = j == NJ - 1
                ot = None if staged else outp.tile([128, b], BF16, tag="ot")
                for h in (0, 1):
                    ps = cpsum.tile([128, 1024], F32, tag="mmps")
                    for t in (2 * h, 2 * h + 1):
                        if j == 0 and t > 0:
                            # bridge the DMA-paced gaps of the first chunk so
                            # the PE never idles long enough to re-throttle
                            dummy_mms((8, 22, 16)[t - 1])
                        for k in range(kc):
                            nc.tensor.matmul(
                                out=ps[:, (t % 2) * 512:(t % 2) * 512 + 512],
                                lhsT=lhs(k),
                                rhs=embt[:, k, t * 512:(t + 1) * 512],
                                start=(k == 0),
                                stop=(k == kc - 1),
                            )
                        if last:
                            # final chunk: evacuate + store 512-col pieces as
                            # each accumulation group finishes, so only one
                            # small piece remains after the last matmul
                            # (shortens the kernel tail by ~3us)
                            p0 = (t % 2) * 512
                            nc.vector.scalar_tensor_tensor(
                                out=ot[:, t * 512:(t + 1) * 512],
                                in0=ps[:, p0:p0 + 512],
                                scalar=rn[:, j:j + 1],
                                in1=renb[:, t * 512:(t + 1) * 512],
                                op0=ALU.mult, op1=ALU.mult,
                            )
                            nc.sync.dma_start(
                                out=outv[j * 128:(j + 1) * 128,
                                         t * 512:(t + 1) * 512],
                                in_=ot[:, t * 512:(t + 1) * 512],
                            )
                    if last:
                        pass
                    elif staged:
                        # rn-only evacuation into SBUF staging (no renb dep)
                        if h == 0:
                            nc.vector.tensor_scalar_mul(
                                out=stg[:, j, 0:1024], in0=ps[:, :],
                                scalar1=rn[:, j:j + 1],
                            )
                        else:
                            nc.scalar.mul(
                                out=stg[:, j, 1024:2048], in_=ps[:, :],
                                mul=rn[:, j:j + 1],
                            )
                    elif h == 0:
                        # ot = (ps * rn[c]) * renb[b], fused on DVE
                        nc.vector.scalar_tensor_tensor(
                            out=ot[:, :1024], in0=ps[:, :], scalar=rn[:, j:j + 1],
                            in1=renb[:, 0:1024], op0=ALU.mult, op1=ALU.mult,
                        )
                    else:
                        ot1 = scrp.tile([128, 1024], BF16, tag="ot1")
                        nc.scalar.mul(
                            out=ot1[:], in_=ps[:, :], mul=rn[:, j:j + 1]
                        )
                        nc.vector.tensor_tensor(
                            out=ot[:, 1024:], in0=ot1[:], in1=renb[:, 1024:2048],
                            op=ALU.mult,
                        )
                if not staged and not last:
                    nc.sync.dma_start(
                        out=outv[j * 128:(j + 1) * 128, :], in_=ot[:]
                    )

                # finalize one staged chunk every other j once renb exists
                if j >= 9 and j % 2 == 1 and dsent < JD:
                    jd = dsent
                    otd = outp.tile([128, b], BF16, tag="ot")
                    nc.vector.tensor_tensor(
                        out=otd[:, 0:1024], in0=stg[:, jd, 0:1024],
                        in1=renb[:, 0:1024], op=ALU.mult,
                    )
                    nc.vector.tensor_tensor(
                        out=otd[:, 1024:2048], in0=stg[:, jd, 1024:2048],
                        in1=renb[:, 1024:2048], op=ALU.mult,
                    )
                    nc.sync.dma_start(
                        out=outv[jd * 128:(jd + 1) * 128, :], in_=otd[:]
                    )
                    dsent += 1

                # wnorm pacing LAST: if a wn DMA runs late it must only
                # delay the (slack-rich) norm chain, never sit ahead of the
                # scalar-side PSUM evacuation in the scalar queue
                while wdone < min(NJ, j + 9):
                    wnorm_chunk(wdone)
                    wdone += 1
                    if wdone % 4 == 0 or wdone == NJ:
                        rn_fin((wdone - 1) // 4)

    nc.compile()
    return nc


_CACHE = {}


def _get_program():
    if "nc" not in _CACHE:
        _CACHE["nc"] = build_program()
    return _CACHE["nc"]


def make_in_maps(embeddings, labels, weight):
    emb = np.asarray(embeddings, dtype=np.float32)
    w = np.asarray(weight, dtype=np.float32)
    labels_np = np.asarray(labels).astype(np.int64)
    embt_bf = np.ascontiguousarray(emb.astype(NPBF).T)
    w_bf = w.astype(NPBF)
    in_maps = []
    perms = []
    at_pos_l = []
    for k in range(N_CORES):
        own = (labels_np // CS) == k
        rows = np.nonzero(own)[0]
        col = labels_np[rows] - k * CS           # class idx within shard
        tc, inv = np.unique(col, return_inverse=True)
        nu, nr = len(tc), len(rows)
        assert nr <= NG * 128, f"core {k}: {nr} owned rows > {NG * 128}"
        # permute the shard so class tc[i] sits at position i (front chunks)
        at_pos = np.arange(CS, dtype=np.int64)   # position -> class
        pos_of = np.arange(CS, dtype=np.int64)   # class -> position
        for i in range(nu):
            c = tc[i]
            jpos = pos_of[c]
            a = at_pos[i]
            at_pos[i], at_pos[jpos] = c, a
            pos_of[c], pos_of[a] = i, jpos
        wn = np.zeros((CSP, D), NPBF)
        wn[:CS] = w_bf[k * CS:(k + 1) * CS]
        touched = np.nonzero(at_pos != np.arange(CS))[0]
        wn[touched] = w_bf[k * CS + at_pos[touched]]
        wT = np.ascontiguousarray(wn.T)
        # gather slots: rows sorted by class position; slot s -> column
        # s//128, partition s%128.  Columns 0..NG-1 index into gsrc
        # (position*B + row), columns NG..2NG-1 index into the renb row.
        order = np.argsort(inv, kind="stable")
        perm = np.full((128, NG), -1, np.int64)
        soff = np.full((128, 2 * NG), OOB, np.int64)
        for s, ri in enumerate(order):
            q, p = divmod(s, 128)
            perm[p, q] = rows[ri]
            soff[p, q] = inv[ri] * B + rows[ri]
            soff[p, NG + q] = rows[ri]
        soff_arr = np.ascontiguousarray(soff.astype(np.int32))
        in_maps.append(
            {"embt": embt_bf, "wt": wT, "wn": wn, "soff": soff_arr}
        )
        perms.append(perm)
        at_pos_l.append(at_pos)
    return in_maps, perms, at_pos_l


def _gather(results, labels, perms, at_pos_l):
    labels_np = np.asarray(labels).astype(np.int64)
    fullT = np.empty((C, B), np.float32)
    ar = np.arange(CS)
    for k in range(N_CORES):
        shard = np.asarray(results[k]["out"]).reshape(CSP, B)
        fullT[k * CS:(k + 1) * CS] = shard[:CS]
        # un-permute the swapped class rows (identity except ~2*nu rows)
        at_pos = at_pos_l[k]
        touched = np.nonzero(at_pos != ar)[0]
        fullT[k * CS + at_pos[touched]] = shard[touched]
        # place the device-computed 64*phi values at the target positions
        tv = np.asarray(results[k]["tv"]).astype(np.float32)  # [128, NG]
        perm = perms[k]
        pp, qq = np.nonzero(perm >= 0)
        rr = perm[pp, qq]
        fullT[labels_np[rr], rr] = tv[pp, qq]
    return fullT.T


def kernel(embeddings, labels, weight):
    nc = _get_program()
    in_maps, perms, at_pos_l = make_in_maps(embeddings, labels, weight)
    res = run_bass_kernel_spmd(nc, in_maps, core_ids=list(range(N_CORES)))
    return _gather(res.results, labels, perms, at_pos_l)


def kernel_profiled(embeddings, labels, weight, **kw):
    """Like kernel() but also returns the BassKernelResults (exec_time_ns)."""
    nc = _get_program()
    in_maps, perms, at_pos_l = make_in_maps(embeddings, labels, weight)
    res = run_bass_kernel_spmd(
        nc, in_maps, core_ids=list(range(N_CORES)), trace=True, **kw
    )
    return _gather(res.results, labels, perms, at_pos_l), res


# revision 36
# speedup vs baseline: 1.0262x; 1.0262x over previous
"""ArcMargin head (ArcFace) distributed over 8 TRN2 NeuronCores.

Strategy (classification / tensor parallel), v11:
  - weight [C, D] sharded along C (12500 classes/core, padded to 12544);
    embeddings + labels replicated.  Weight is uploaded bf16 twice
    (transposed [D, CSP] for the matmul lhsT, natural [CSP, D] for the class
    norms); embeddings only once, transposed [D, B] (raw matmul rhs).
  - TRANSPOSED logits out[c, b] = 64 * (w_c . e_hat_b): classes sit on PSUM
    partitions, so both norms fold into PSUM evacuation: 1/||w_c|| is a
    per-partition scalar, 64/||e_b|| is the per-column tensor renb [128, B],
    built on device as ones[128,128] @ embt^2 (a K=128 matmul computes all
    column sums-of-squares broadcast to every partition) + sqrt + recip.
  - The first JD=8 chunks evacuate into SBUF staging with only the rn scale
    (renb is not ready yet) and are finalized a few chunks later - the
    TensorEngine never waits on the norm chain.  Scheduling rules learned
    from traces: cross-queue DMA dependencies act as per-queue completion
    barriers (DMA issue order = need order), a PE instruction must never
    wait on a SWDGE (gpsimd-queue) DMA semaphore, an indirect (SWDGE)
    gather must never be followed by writes to the tensor it reads (the
    WAR dependency bubbles the whole write queue behind it), and the
    finalize window j=9..23 runs the DMA queue at ~350 GB/s - adding any
    extra traffic there backlogs the weight prefetches and stalls the PE.
  - Output is bf16 (halves the dominant HBM write traffic; rel-err budget
    2e-2 >> bf16 noise).
  - ArcFace margin (v11): the host PERMUTES each core's class shard so that
    every target class (the ones needing the margin) sits in the first
    NG*128 positions = the first staged chunks.  Those chunks' rn-scaled
    staging values are copied to a dedicated DRAM buffer gsrc at j=1..4
    (a window where the store queue is otherwise idle), and renb row 0 is
    dumped to DRAM (4 KB) right after it's built.  At j=6/7 two tiny SWDGE
    gathers fetch the staged value and its renb factor for each target row;
    at j=10 the DVE reconstructs 64*cos = stg*renb, applies the phi
    formula, and ships 64*phi in a tiny tensor.  The whole margin chain
    lives in j<=11, touches nothing the main stream depends on, and
    NOTHING runs after the last chunk's store.  The host un-permutes
    during unshard (indexing only, ~500 rows).
  - Prologue (v11): embeddings stream in 4x512-column pieces interleaved
    with the first weight blocks so matmuls can start on a 0.75 MB
    prefix; ~36 dummy 128-column matmuls on the ones tile warm the PE HAM
    clock-gate (4/8 -> 8/8 after ~3.4us of activity) before the stream,
    and more dummies bridge chunk 0's DMA-paced gaps so the HAM MID
    window never re-throttles.
"""

import math
import sys

import numpy as np
import ml_dtypes

for _p in ("/opt/trn_rl_repo",):
    if _p not in sys.path:
        sys.path.append(_p)

import concourse.bass as bass
import concourse.tile as tile
from concourse import bacc
from concourse import mybir
from concourse.bass_utils import run_bass_kernel_spmd

SCALE = 64.0
MARGIN = 0.5
COS_M = math.cos(MARGIN)
SIN_M = math.sin(MARGIN)
TH = math.cos(math.pi - MARGIN)
MM = math.sin(math.pi - MARGIN) * MARGIN

B, D, C = 2048, 512, 100000
N_CORES = 8
CS = C // N_CORES          # 12500 real classes per core
CSP = 12544                # padded classes per core (98 * 128)
NJ = CSP // 128            # 98 class chunks
CB = 1792                  # weight-block width (7 blocks x 14 chunks)
NBLK = CSP // CB           # 7
JPB = CB // 128            # 14 chunks per block
OOB = 1 << 30              # gather offset sentinel for "not my row"
JD = 8                     # chunks evacuated to SBUF staging (pre-renb)
NG = 4                     # phi gather columns (target classes live in
                           # chunks 0..NG-1 after the host permutation)
NWARM = 36                 # HAM warm-up dummy matmuls (N=128 each)

NPBF = ml_dtypes.bfloat16

F32 = mybir.dt.float32
BF16 = mybir.dt.bfloat16
I32 = mybir.dt.int32
AF = mybir.ActivationFunctionType
ALU = mybir.AluOpType


def build_program(b=B, d=D, csp=CSP):
    """Build the (SPMD-uniform) single-core Bass program."""
    kc = d // 128          # 4 contraction chunks
    nc = bacc.Bacc()

    embt_d = nc.declare_dram_parameter("embt", [d, b], BF16, isOutput=False)
    wt_d = nc.declare_dram_parameter("wt", [d, csp], BF16, isOutput=False)
    wn_d = nc.declare_dram_parameter("wn", [csp, d], BF16, isOutput=False)
    soff_d = nc.declare_dram_parameter(
        "soff", [128, 2 * NG], I32, isOutput=False
    )
    # flat transposed output [c * B + b]
    out_d = nc.declare_dram_parameter("out", [csp * b, 1], BF16, isOutput=True)
    tv_d = nc.declare_dram_parameter("tv", [128, NG], F32, isOutput=True)
    # phi gather sources: staged (rn-scaled, pre-renb) copies of chunks
    # 0..NG-1, and one row of renb.  Both written in the idle early window
    # and only ever READ afterwards, so the gathers carry no WAR hazard
    # against anything.
    gsrc_d = nc.declare_dram_parameter(
        "gsrc", [NG * 128 * b, 1], BF16, isOutput=True
    )
    renb_d = nc.declare_dram_parameter("renbd", [b, 1], BF16, isOutput=True)

    with tile.TileContext(nc) as tc:
        with (
            tc.tile_pool(name="const", bufs=1) as constp,
            tc.tile_pool(name="persist", bufs=1) as persist,
            tc.tile_pool(name="wtp", bufs=3) as wtp,
            tc.tile_pool(name="wnp", bufs=3) as wnp,
            tc.tile_pool(name="scr", bufs=2) as scrp,
            tc.tile_pool(name="smp", bufs=4) as smp,
            tc.tile_pool(name="outp", bufs=4) as outp,
            tc.tile_pool(name="stg", bufs=1) as stgp,
            tc.tile_pool(name="cpsum", bufs=4, space="PSUM") as cpsum,
        ):
            zb = constp.tile([128, 1], F32, tag="zb")
            nc.vector.memset(zb[:], 0.0)
            epsb = constp.tile([128, 1], F32, tag="epsb")
            nc.vector.memset(epsb[:], 1e-24)
            s2b = constp.tile([128, 1], F32, tag="s2b")
            nc.vector.memset(s2b[:], SCALE * SCALE)
            onesb = constp.tile([128, 128], BF16, tag="onesb")
            nc.vector.memset(onesb[:], 1.0)

            embt = persist.tile([128, kc, b], BF16)     # e^T raw (matmul rhs)
            sqt = persist.tile([128, kc, b], BF16)      # embt^2
            renb = persist.tile([128, b], BF16)         # 64/||e_b|| bcast
            rsf = persist.tile([128, b], F32)           # 1/sum(e^2) scratch
            nsq = persist.tile([128, NJ], F32)          # per-class sum(w^2)
            nrm = persist.tile([128, NJ], F32)
            rn = persist.tile([128, NJ], F32)           # 1/||w_c||
            svec = persist.tile([128, NG], BF16)        # stg value of targets
            renbg = persist.tile([128, NG], BF16)       # renb value of targets
            tval = persist.tile([128, NG], F32)         # 64*phi, sorted
            sofft = persist.tile([128, 2 * NG], I32)
            stg = stgp.tile([128, JD, b], BF16)         # staged rn-scaled out

            outv = out_d[:].rearrange("(c b) o -> c (b o)", b=b)  # [csp, b]
            gsv = gsrc_d[:].rearrange("(c b) o -> c (b o)", b=b)  # [NG*128, b]

            # ---------------- DMA helpers ----------------
            wt_tiles = {}

            def wt_blk(blk):
                t = wtp.tile([128, kc, CB], BF16, tag="wt", name=f"wt_{blk}")
                nc.sync.dma_start(
                    out=t[:],
                    in_=wt_d[:, blk * CB:(blk + 1) * CB].rearrange(
                        "(k p) c -> p k c", p=128
                    ),
                )
                wt_tiles[blk] = t

            wn_tiles = {}

            def wn_g(g):
                r0 = g * 512
                ng = min(4, NJ - g * 4)
                t = wnp.tile([128, 4, d], BF16, tag="wn", name=f"wn_{g}")
                nc.sync.dma_start(
                    out=t[:, :ng, :],
                    in_=wn_d[r0:r0 + ng * 128, :].rearrange(
                        "(g2 p) dd -> p g2 dd", p=128
                    ),
                )
                wn_tiles[g] = t

            # ---------------- compute helpers ----------------
            def wnorm_chunk(c):
                sq = scrp.tile([128, d], BF16, tag="sqw")
                nc.scalar.activation(
                    out=sq[:], in_=wn_tiles[c // 4][:, c % 4, :], func=AF.Square,
                    bias=zb[:], accum_out=nsq[:, c:c + 1],
                )

            def rn_fin(g):
                s0 = g * 4
                s1 = min(s0 + 4, NJ)
                nc.scalar.activation(
                    out=nrm[:, s0:s1], in_=nsq[:, s0:s1], func=AF.Sqrt, bias=epsb[:]
                )
                nc.vector.reciprocal(out=rn[:, s0:s1], in_=nrm[:, s0:s1])

            def phi_gather():
                # all of gsrc is written by j=5; nothing writes it again, so
                # these gathers block nothing.  Per-column [128,1] offsets
                # (multi-column offset APs scramble the columns).
                for q in range(NG):
                    nc.gpsimd.indirect_dma_start(
                        out=svec[:, q:q + 1],
                        out_offset=None,
                        in_=gsrc_d[:],
                        in_offset=bass.IndirectOffsetOnAxis(
                            ap=sofft[:, q:q + 1], axis=0
                        ),
                        bounds_check=NG * 128 * b - 1,
                        oob_is_err=False,
                    )

            def renb_gather():
                for q in range(NG):
                    nc.gpsimd.indirect_dma_start(
                        out=renbg[:, q:q + 1],
                        out_offset=None,
                        in_=renb_d[:],
                        in_offset=bass.IndirectOffsetOnAxis(
                            ap=sofft[:, NG + q:NG + q + 1], axis=0
                        ),
                        bounds_check=b - 1,
                        oob_is_err=False,
                    )

            def phi_block():
                # 64*cos = staged value * renb factor (both gathered)
                sb = smp.tile([128, NG], F32, tag="sb")
                nc.vector.tensor_tensor(
                    out=sb[:], in0=svec[:, :], in1=renbg[:, :], op=ALU.mult
                )
                s2 = smp.tile([128, NG], F32, tag="s2")
                nc.vector.tensor_tensor(out=s2[:], in0=sb[:], in1=sb[:],
                                        op=ALU.mult)
                sn = smp.tile([128, NG], F32, tag="sn")
                # sin = sqrt(4096 - s^2); s^2 <= 4096 exactly (|cos| <= 1)
                nc.scalar.activation(
                    out=sn[:], in_=s2[:], func=AF.Sqrt, bias=s2b[:], scale=-1.0
                )
                pc = smp.tile([128, NG], F32, tag="pc")
                nc.vector.tensor_scalar_mul(out=pc[:], in0=sb[:], scalar1=COS_M)
                smt = smp.tile([128, NG], F32, tag="smt")
                nc.vector.tensor_scalar_mul(out=smt[:], in0=sn[:], scalar1=SIN_M)
                ph = smp.tile([128, NG], F32, tag="ph")
                nc.vector.tensor_tensor(
                    out=ph[:], in0=pc[:], in1=smt[:], op=ALU.subtract
                )
                eb = smp.tile([128, NG], F32, tag="eb")
                nc.vector.tensor_scalar_add(
                    out=eb[:], in0=sb[:], scalar1=-SCALE * MM
                )
                mk = smp.tile([128, NG], mybir.dt.uint8, tag="mk")
                nc.vector.tensor_scalar(
                    out=mk[:], in0=sb[:], scalar1=SCALE * TH, scalar2=None,
                    op0=ALU.is_gt,
                )
                nc.vector.select(
                    out=tval[:, :], mask=mk[:], on_true=ph[:], on_false=eb[:]
                )
                nc.sync.dma_start(out=tv_d[:, :], in_=tval[:, :])

            # ---------------- prologue (DMA order = ring order; order sets
            # the completion barrier each consumer waits on) ----------------
            def embt_q(qq):
                nc.sync.dma_start(
                    out=embt[:, :, qq * 512:(qq + 1) * 512],
                    in_=embt_d[:, qq * 512:(qq + 1) * 512].rearrange(
                        "(k p) c -> p k c", p=128
                    ),
                )

            embt_q(0)
            wt0a = wtp.tile([128, kc, 256], BF16, tag="wt0a")
            nc.sync.dma_start(
                out=wt0a[:],
                in_=wt_d[:, 0:256].rearrange("(k p) c -> p k c", p=128),
            )
            embt_q(1)
            wn_g(0)
            embt_q(2)
            embt_q(3)
            wt0b = wtp.tile([128, kc, 768], BF16, tag="wt0b")
            nc.sync.dma_start(
                out=wt0b[:],
                in_=wt_d[:, 256:1024].rearrange("(k p) c -> p k c", p=128),
            )
            wt0c = wtp.tile([128, kc, CB - 1024], BF16, tag="wt0c")
            nc.sync.dma_start(
                out=wt0c[:],
                in_=wt_d[:, 1024:CB].rearrange("(k p) c -> p k c", p=128),
            )
            wn_g(1)
            wn_g(2)
            wt_blk(1)
            nc.sync.dma_start(out=sofft[:], in_=soff_d[:])

            # HAM warm-up: N=128 dummy matmuls on the ones tile keep the
            # PE busy from ~6.3us (constants ready) while the first real
            # operands stream in, flipping the clock gate to 8/8 before the
            # real matmul stream begins.  More dummies are interleaved into
            # chunk 0 (below) to bridge the DMA-paced gaps so the HAM MID
            # window never sees enough idle to re-throttle.
            wmps = cpsum.tile([128, 1024], F32, tag="mmps", name="warm")

            def dummy_mms(n):
                for _ in range(n):
                    nc.tensor.matmul(
                        out=wmps[:, 0:128], lhsT=onesb[:], rhs=onesb[:],
                        start=True, stop=True,
                    )

            dummy_mms(NWARM)

            wdone = 0
            while wdone < 8:
                wnorm_chunk(wdone)
                wdone += 1
                if wdone % 4 == 0:
                    rn_fin(wdone // 4 - 1)

            # ---------------- main loop over class chunks ----------------
            dsent = 0
            for j in range(NJ):
                blk, jj = divmod(j, JPB)
                # two-block weight prefetch (bufs=3: cur, +1, +2 in flight)
                if j == 0:
                    wt_blk(2)
                elif jj == 0 and 1 <= blk <= NBLK - 3:
                    wt_blk(blk + 2)
                if j % 4 == 0:
                    g = j // 4 + 3
                    if g * 4 < NJ:
                        wn_g(g)
                while wdone < min(NJ, j + 9):
                    wnorm_chunk(wdone)
                    wdone += 1
                    if wdone % 4 == 0 or wdone == NJ:
                        rn_fin((wdone - 1) // 4)

                # staged chunks 0..NG-1 -> gsrc (idle store-queue window)
                if 1 <= j <= NG:
                    nc.sync.dma_start(
                        out=gsv[(j - 1) * 128:j * 128, :], in_=stg[:, j - 1, :]
                    )

                if j == 3:
                    for k in range(kc):
                        nc.vector.tensor_tensor(
                            out=sqt[:, k, :], in0=embt[:, k, :],
                            in1=embt[:, k, :], op=ALU.mult,
                        )
                if j == 4:
                    # renb = 64/||e_b|| on every partition: ones-matmul
                    # computes column sums of embt^2 broadcast over partitions
                    for hh in range(2):
                        psr = cpsum.tile(
                            [128, 1024], F32, tag="mmps", name=f"ps_re{hh}"
                        )
                        for t2 in range(2):
                            for k in range(kc):
                                nc.tensor.matmul(
                                    out=psr[:, t2 * 512:(t2 + 1) * 512],
                                    lhsT=onesb[:],
                                    rhs=sqt[:, k, (2 * hh + t2) * 512:
                                            (2 * hh + t2 + 1) * 512],
                                    start=(k == 0),
                                    stop=(k == kc - 1),
                                )
                        # renb = 64/sqrt(ssq) = sqrt(4096 * (1/ssq)):
                        # fast approx reciprocal (18 bits >> bf16), then a
                        # scalar-engine sqrt with the scale folded in
                        nc.vector.reciprocal_approx_fast(
                            out=rsf[:, hh * 1024:(hh + 1) * 1024], in_=psr[:]
                        )
                        nc.scalar.activation(
                            out=renb[:, hh * 1024:(hh + 1) * 1024],
                            in_=rsf[:, hh * 1024:(hh + 1) * 1024],
                            func=AF.Sqrt, bias=epsb[:], scale=SCALE * SCALE,
                        )
                if j == 5:
                    # one row of renb to DRAM (4 KB) for the target gather
                    nc.sync.dma_start(
                        out=renb_d[:].rearrange("b o -> o b"),
                        in_=renb[0:1, :],
                    )
                if j == 6:
                    phi_gather()
                if j == 7:
                    renb_gather()
                if j == 30:
                    # late enough that the (slow, serialized) SWDGE gathers
                    # are long finished - the DVE queue must never block on
                    # them (head-of-line: every evac behind it would stall
                    # and backpressure PSUM into the PE)
                    phi_block()

                if blk == 0:
                    def lhs(k, jj=jj):
                        if jj < 2:
                            return wt0a[:, k, jj * 128:(jj + 1) * 128]
                        if jj < 8:
                            return wt0b[:, k, (jj - 2) * 128:(jj - 1) * 128]
                        return wt0c[:, k, (jj - 8) * 128:(jj - 7) * 128]
                else:
                    def lhs(k, jj=jj, cw=wt_tiles[blk]):
                        return cw[:, k, jj * 128:(jj + 1) * 128]

                staged = j < JD
                last = j == NJ - 1
                ot = None if staged else outp.tile([128, b], BF16, tag="ot")
                for h in (0, 1):
                    ps = cpsum.tile([128, 1024], F32, tag="mmps")
                    for t in (2 * h, 2 * h + 1):
                        if j == 0 and t > 0:
                            # bridge the DMA-paced gaps of the first chunk so
                            # the PE never idles long enough to re-throttle
                            dummy_mms((8, 20, 12)[t - 1])
                        for k in range(kc):
                            nc.tensor.matmul(
                                out=ps[:, (t % 2) * 512:(t % 2) * 512 + 512],
                                lhsT=lhs(k),
                                rhs=embt[:, k, t * 512:(t + 1) * 512],
                                start=(k == 0),
                                stop=(k == kc - 1),
                            )
                        if last:
                            # final chunk: evacuate + store 512-col pieces as
                            # each accumulation group finishes, so only one
                            # small piece remains after the last matmul
                            # (shortens the kernel tail by ~3us)
                            p0 = (t % 2) * 512
                            nc.vector.scalar_tensor_tensor(
                                out=ot[:, t * 512:(t + 1) * 512],
                                in0=ps[:, p0:p0 + 512],
                                scalar=rn[:, j:j + 1],
                                in1=renb[:, t * 512:(t + 1) * 512],
                                op0=ALU.mult, op1=ALU.mult,
                            )
                            nc.sync.dma_start(
                                out=outv[j * 128:(j + 1) * 128,
                                         t * 512:(t + 1) * 512],
                                in_=ot[:, t * 512:(t + 1) * 512],
                            )
                    if last:
                        pass
                    elif staged:
                        # rn-only evacuation into SBUF staging (no renb dep)
                        if h == 0:
                            nc.vector.tensor_scalar_mul(
                                out=stg[:, j, 0:1024], in0=ps[:, :],
                                scalar1=rn[:, j:j + 1],
                            )
                        else:
                            nc.scalar.mul(
                                out=stg[:, j, 1024:2048], in_=ps[:, :],
                                mul=rn[:, j:j + 1],
                            )
                    elif h == 0:
                        # ot = (ps * rn[c]) * renb[b], fused on DVE
                        nc.vector.scalar_tensor_tensor(
                            out=ot[:, :1024], in0=ps[:, :], scalar=rn[:, j:j + 1],
                            in1=renb[:, 0:1024], op0=ALU.mult, op1=ALU.mult,
                        )
                    else:
                        ot1 = scrp.tile([128, 1024], BF16, tag="ot1")
                        nc.scalar.mul(
                            out=ot1[:], in_=ps[:, :], mul=rn[:, j:j + 1]
                        )
                        nc.vector.tensor_tensor(
                            out=ot[:, 1024:], in0=ot1[:], in1=renb[:, 1024:2048],
                            op=ALU.mult,
                        )
                if not staged and not last:
                    nc.sync.dma_start(
                        out=outv[j * 128:(j + 1) * 128, :], in_=ot[:]
                    )

                # finalize one staged chunk every other j once renb exists
                if j >= 9 and j % 2 == 1 and dsent < JD:
                    jd = dsent
                    otd = outp.tile([128, b], BF16, tag="ot")
                    nc.vector.tensor_tensor(
                        out=otd[:, 0:1024], in0=stg[:, jd, 0:1024],
                        in1=renb[:, 0:1024], op=ALU.mult,
                    )
                    nc.vector.tensor_tensor(
                        out=otd[:, 1024:2048], in0=stg[:, jd, 1024:2048],
                        in1=renb[:, 1024:2048], op=ALU.mult,
                    )
                    nc.sync.dma_start(
                        out=outv[jd * 128:(jd + 1) * 128, :], in_=otd[:]
                    )
                    dsent += 1

    nc.compile()
    return nc


_CACHE = {}


def _get_program():
    if "nc" not in _CACHE:
        _CACHE["nc"] = build_program()
    return _CACHE["nc"]


def make_in_maps(embeddings, labels, weight):
    emb = np.asarray(embeddings, dtype=np.float32)
    w = np.asarray(weight, dtype=np.float32)
    labels_np = np.asarray(labels).astype(np.int64)
    embt_bf = np.ascontiguousarray(emb.astype(NPBF).T)
    w_bf = w.astype(NPBF)
    in_maps = []
    perms = []
    at_pos_l = []
    for k in range(N_CORES):
        own = (labels_np // CS) == k
        rows = np.nonzero(own)[0]
        col = labels_np[rows] - k * CS           # class idx within shard
        tc, inv = np.unique(col, return_inverse=True)
        nu, nr = len(tc), len(rows)
        assert nr <= NG * 128, f"core {k}: {nr} owned rows > {NG * 128}"
        # permute the shard so class tc[i] sits at position i (front chunks)
        at_pos = np.arange(CS, dtype=np.int64)   # position -> class
        pos_of = np.arange(CS, dtype=np.int64)   # class -> position
        for i in range(nu):
            c = tc[i]
            jpos = pos_of[c]
            a = at_pos[i]
            at_pos[i], at_pos[jpos] = c, a
            pos_of[c], pos_of[a] = i, jpos
        wn = np.zeros((CSP, D), NPBF)
        wn[:CS] = w_bf[k * CS:(k + 1) * CS]
        touched = np.nonzero(at_pos != np.arange(CS))[0]
        wn[touched] = w_bf[k * CS + at_pos[touched]]
        wT = np.ascontiguousarray(wn.T)
        # gather slots: rows sorted by class position; slot s -> column
        # s//128, partition s%128.  Columns 0..NG-1 index into gsrc
        # (position*B + row), columns NG..2NG-1 index into the renb row.
        order = np.argsort(inv, kind="stable")
        perm = np.full((128, NG), -1, np.int64)
        soff = np.full((128, 2 * NG), OOB, np.int64)
        for s, ri in enumerate(order):
            q, p = divmod(s, 128)
            perm[p, q] = rows[ri]
            soff[p, q] = inv[ri] * B + rows[ri]
            soff[p, NG + q] = rows[ri]
        soff_arr = np.ascontiguousarray(soff.astype(np.int32))
        in_maps.append(
            {"embt": embt_bf, "wt": wT, "wn": wn, "soff": soff_arr}
        )
        perms.append(perm)
        at_pos_l.append(at_pos)
    return in_maps, perms, at_pos_l


def _gather(results, labels, perms, at_pos_l):
    labels_np = np.asarray(labels).astype(np.int64)
    fullT = np.empty((C, B), np.float32)
    ar = np.arange(CS)
    for k in range(N_CORES):
        shard = np.asarray(results[k]["out"]).reshape(CSP, B)
        fullT[k * CS:(k + 1) * CS] = shard[:CS]
        # un-permute the swapped class rows (identity except ~2*nu rows)
        at_pos = at_pos_l[k]
        touched = np.nonzero(at_pos != ar)[0]
        fullT[k * CS + at_pos[touched]] = shard[touched]
        # place the device-computed 64*phi values at the target positions
        tv = np.asarray(results[k]["tv"]).astype(np.float32)  # [128, NG]
        perm = perms[k]
        pp, qq = np.nonzero(perm >= 0)
        rr = perm[pp, qq]
        fullT[labels_np[rr], rr] = tv[pp, qq]
    return fullT.T


def kernel(embeddings, labels, weight):
    nc = _get_program()
    in_maps, perms, at_pos_l = make_in_maps(embeddings, labels, weight)
    res = run_bass_kernel_spmd(nc, in_maps, core_ids=list(range(N_CORES)))
    return _gather(res.results, labels, perms, at_pos_l)


def kernel_profiled(embeddings, labels, weight, **kw):
    """Like kernel() but also returns the BassKernelResults (exec_time_ns)."""
    nc = _get_program()
    in_maps, perms, at_pos_l = make_in_maps(embeddings, labels, weight)
    res = run_bass_kernel_spmd(
        nc, in_maps, core_ids=list(range(N_CORES)), trace=True, **kw
    )
    return _gather(res.results, labels, perms, at_pos_l), res
